# revision 3
# baseline (speedup 1.0000x reference)
"""AdaptiveDiffusionLayer on 8 TRN2 NeuronCores.

out = (1 - t) * support + t * (adj @ support),  support = x @ weight

Strategy (transposed 1D column-parallel SpMM + feature-sharded ReduceScatter):
  - Fold the identity mix into the matrix on the host: A' = t*adj + (1-t)*I,
    so the device computes a pure A' @ support.
  - Column-shard A' across 8 cores (contraction dim k): core c owns
    A'[:, c*1250:(c+1)*1250]. Shard x by the same k rows so support_c =
    x_c @ W is entirely local.
  - Compute the TRANSPOSED output: out^T[f, i] = sum_k sup[k, f] * A'[i, k].
    The stationary PE operand is a support tile [125k x 128f] (reused for a
    whole 2000-column stream of adj -> ~10x fewer LDWEIGHTS than making adj
    stationary), the moving operand is A'^T streamed straight from HBM.
  - adj is host-packed per core as [5 groups][125 part][10 q][2000 i] so each
    group loads with two 2.5MB dma_starts (125 x 20KB descriptors each), one
    on each HWDGE queue (sync + scalar), spreading across the SDMA engines.
  - Partials [512f x 2000i] per group feed a pipelined ReduceScatter over the
    feature dim (512 = 8 ranks x 64); compute of group g+1 overlaps the
    collective of group g. Each core ends with out^T[64c:64c+64, :]; the host
    concatenates and transposes.
"""

import sys

for _p in ("/opt/trn_rl_repo",):
    if _p not in sys.path:
        sys.path.append(_p)

import numpy as np
import ml_dtypes

from concourse import bass, bacc, mybir, tile
from concourse.bass_utils import run_bass_kernel_spmd

N = 10000
IN_F = 512
OUT_F = 512
C = 8               # cores
R = N // C          # 1250 k rows per core
KT = 125            # k-tile (PE contraction rows)
NQ = R // KT        # 10 k-tiles
G = 5               # i groups
IG = N // G         # 2000 output rows (i) per group
IC = 500            # psum chunk (columns per PSUM bank)
NIC = IG // IC      # 4 chunks per group
FJ = 128            # feature chunk (PE stationary free dim)
NJ = OUT_F // FJ    # 4 feature chunks
NI = IN_F // 128    # 4 support contraction chunks
OC = OUT_F // C     # 64 feature rows per rank after ReduceScatter

BF16 = mybir.dt.bfloat16
F32 = mybir.dt.float32

_cached = {}


def _build():
    nc = bacc.Bacc("TRN2", target_bir_lowering=False, debug=False, num_devices=C)

    adjp = nc.dram_tensor("adjp", [G, KT, NQ * IG], BF16, kind="ExternalInput")
    xt = nc.dram_tensor("xt", [IN_F, R], BF16, kind="ExternalInput")
    w = nc.dram_tensor("w", [IN_F, OUT_F], BF16, kind="ExternalInput")
    out = nc.dram_tensor("out", [OC, N], BF16, kind="ExternalOutput")

    rs_in = [nc.dram_tensor(f"rs_in{g}", [OUT_F, IG], BF16) for g in range(G)]
    rs_out = [nc.dram_tensor(f"rs_out{g}", [OC, IG], BF16) for g in range(G)]

    with tile.TileContext(nc) as tc:
        with (
            tc.tile_pool(name="persist", bufs=1) as p_pers,
            tc.tile_pool(name="sup", bufs=1) as p_sup,
            tc.tile_pool(name="slab", bufs=2) as p_slab,
            tc.tile_pool(name="stage", bufs=4) as p_stage,
        ):
            xt_sb = p_pers.tile([128, NI * R], BF16, tag="xt_sb", name="xt_sb")
            w_sb = p_pers.tile([128, NI * OUT_F], BF16, tag="w_sb", name="w_sb")
            for j in range(NI):
                nc.scalar.dma_start(
                    out=xt_sb[:, j * R:(j + 1) * R],
                    in_=xt[j * 128:(j + 1) * 128, :],
                )
                nc.scalar.dma_start(
                    out=w_sb[:, j * OUT_F:(j + 1) * OUT_F],
                    in_=w[j * 128:(j + 1) * 128, :],
                )

            # ---- support_c = x_c @ W (all local), kept bf16 as the PE
            # stationary operand for the main SpMM ----
            supbf = []
            with tc.tile_pool(name="psum_sup", bufs=3, space="PSUM") as pp_sup:
                for s in range(NQ):
                    ps = pp_sup.tile([KT, OUT_F], F32, tag="ps", name=f"ps{s}")
                    for j in range(NI):
                        nc.tensor.matmul(
                            ps[:, :],
                            lhsT=xt_sb[:, j * R + s * KT: j * R + (s + 1) * KT],
                            rhs=w_sb[:, j * OUT_F:(j + 1) * OUT_F],
                            start=(j == 0),
                            stop=(j == NI - 1),
                        )
                    sb = p_sup.tile(
                        [KT, OUT_F], BF16, tag=f"supbf{s}", name=f"supbf{s}"
                    )
                    nc.vector.tensor_copy(sb[:, :], ps[:, :])
                    supbf.append(sb)

            # ---- main SpMM, transposed: for each i-group (2000 rows) and
            # feature chunk j (128), accumulate over the 10 local k-tiles.
            # One compound matmul per (g, j, q) writes 4 PSUM banks
            # (4 x 500 cols) off a single weight load. ----
            HALF = NQ * IG // 2
            with tc.tile_pool(name="psum_main", bufs=1, space="PSUM") as pp_main:
                for g in range(G):
                    slab = p_slab.tile(
                        [KT, NQ * IG], BF16, tag="slab", name=f"slab{g}"
                    )
                    nc.sync.dma_start(
                        out=slab[:, 0:HALF], in_=adjp[g, :, 0:HALF]
                    )
                    nc.scalar.dma_start(
                        out=slab[:, HALF:NQ * IG], in_=adjp[g, :, HALF:NQ * IG]
                    )
                    for j in range(NJ):
                        acc = pp_main.tile(
                            [128, NIC * 512], F32, tag=f"acc{j % 2}",
                            name=f"acc{g}_{j}",
                        )
                        for q in range(NQ):
                            for ic in range(NIC):
                                nc.tensor.matmul(
                                    acc[:, ic * 512: ic * 512 + IC],
                                    lhsT=supbf[q][:, j * FJ:(j + 1) * FJ],
                                    rhs=slab[:, q * IG + ic * IC:
                                             q * IG + (ic + 1) * IC],
                                    start=(q == 0),
                                    stop=(q == NQ - 1),
                                )
                        stage = p_stage.tile(
                            [128, IG], BF16, tag="stage", name=f"stage{g}_{j}"
                        )
                        nc.vector.tensor_copy(
                            stage[:, :].rearrange("p (a b) -> p a b", a=NIC),
                            acc[:, :].rearrange(
                                "p (a b) -> p a b", a=NIC
                            )[:, :, 0:IC],
                        )
                        nc.gpsimd.dma_start(
                            out=rs_in[g][j * FJ:(j + 1) * FJ, :],
                            in_=stage[:, :],
                        )
                    nc.gpsimd.collective_compute(
                        "ReduceScatter",
                        mybir.AluOpType.add,
                        replica_groups=[list(range(C))],
                        ins=[rs_in[g].ap().opt()],
                        outs=[rs_out[g].ap().opt()],
                    )
                    nc.scalar.dma_start(
                        out=out[:, g * IG:(g + 1) * IG],
                        in_=rs_out[g][:, :],
                    )

    nc.compile()
    return nc


def _shard_inputs(x, adj, t, weight):
    bf16 = ml_dtypes.bfloat16
    t0 = float(np.asarray(t, np.float32).reshape(-1)[0])
    A = np.asarray(adj, np.float32) * t0
    idx = np.arange(N)
    A[idx, idx] += 1.0 - t0
    Ab = A.astype(bf16)                       # [N(i), N(k)] bf16
    x = np.asarray(x, np.float32)
    w_bf = np.asarray(weight, np.float32).astype(bf16)

    in_maps = []
    for c in range(C):
        cols = slice(c * R, (c + 1) * R)
        blk = Ab[:, cols]                     # [10000 i, 1250 k]
        adjpc = np.ascontiguousarray(
            blk.reshape(G, IG, NQ, KT).transpose(0, 3, 2, 1)
        ).reshape(G, KT, NQ * IG)
        xtc = np.ascontiguousarray(x[cols].T).astype(bf16)  # [IN_F, R]
        in_maps.append({"adjp": adjpc, "xt": xtc, "w": w_bf})
    return in_maps


def _assemble(res):
    outT = np.concatenate(
        [np.asarray(res.results[c]["out"]).astype(np.float32)
         for c in range(C)],
        axis=0,
    )                                         # [512, 10000]
    return np.ascontiguousarray(outT.T)       # [10000, 512]


def kernel(x, adj, t, weight):
    if "nc" not in _cached:
        _cached["nc"] = _build()
    nc = _cached["nc"]
    in_maps = _shard_inputs(x, adj, t, weight)
    res = run_bass_kernel_spmd(nc, in_maps, list(range(C)))
    return _assemble(res)


# revision 10
# speedup vs baseline: 1.0666x; 1.0666x over previous
"""AdaptiveDiffusionLayer on 8 TRN2 NeuronCores.

out = (1 - t) * support + t * (adj @ support),  support = x @ weight

Strategy (transposed 1D column-parallel SpMM + feature-sharded ReduceScatter):
  - Fold the identity mix into the matrix on the host: A' = t*adj + (1-t)*I,
    so the device computes a pure A' @ support.
  - Column-shard A' across 8 cores (contraction dim k): core c owns
    A'[:, c*1250:(c+1)*1250]. Shard x by the same k rows so support_c =
    x_c @ W is entirely local.
  - Compute the TRANSPOSED output: out^T[f, i] = sum_k sup[k, f] * A'[i, k].
    The stationary PE operand is a support tile [125k x 128f] (reused for a
    whole 2000-column stream of adj -> ~10x fewer LDWEIGHTS than making adj
    stationary), the moving operand is A'^T streamed straight from HBM.
  - adj is host-packed per core as [5 groups][125 part][10 q][2000 i] so each
    group loads with two 2.5MB dma_starts (125 x 20KB descriptors each), one
    on each HWDGE queue (sync + scalar), spreading across the SDMA engines.
  - Partials [512f x 2000i] per group feed a pipelined ReduceScatter over the
    feature dim (512 = 8 ranks x 64); compute of group g+1 overlaps the
    collective of group g. Each core ends with out^T[64c:64c+64, :]; the host
    concatenates and transposes.
"""

import sys

for _p in ("/opt/trn_rl_repo",):
    if _p not in sys.path:
        sys.path.append(_p)

import numpy as np
import ml_dtypes

from concourse import bass, bacc, mybir, tile
from concourse.bass_utils import run_bass_kernel_spmd

N = 10000
IN_F = 512
OUT_F = 512
C = 8               # cores
R = N // C          # 1250 k rows per core
KT = 125            # k-tile (PE contraction rows)
NQ = R // KT        # 10 k-tiles
G = 5               # i groups
IG = N // G         # 2000 output rows (i) per group
IC = 500            # psum chunk (columns per PSUM bank)
NIC = IG // IC      # 4 chunks per group
FJ = 128            # feature chunk (PE stationary free dim)
NJ = OUT_F // FJ    # 4 feature chunks
NI = IN_F // 128    # 4 support contraction chunks
OC = OUT_F // C     # 64 feature rows per rank after ReduceScatter

BF16 = mybir.dt.bfloat16
F32 = mybir.dt.float32

_cached = {}


def _build():
    nc = bacc.Bacc("TRN2", target_bir_lowering=False, debug=False, num_devices=C)

    adjp = nc.dram_tensor("adjp", [G, KT, NQ * IG], BF16, kind="ExternalInput")
    xt = nc.dram_tensor("xt", [IN_F, R], BF16, kind="ExternalInput")
    w = nc.dram_tensor("w", [IN_F, OUT_F], BF16, kind="ExternalInput")
    out = nc.dram_tensor("out", [G, OC, IG], BF16, kind="ExternalOutput")

    rs_in = [nc.dram_tensor(f"rs_in{g}", [OUT_F, IG], BF16) for g in range(G)]
    rs_out = [nc.dram_tensor(f"rs_out{g}", [OC, IG], BF16) for g in range(G)]

    with tile.TileContext(nc) as tc:
        with (
            tc.tile_pool(name="persist", bufs=1) as p_pers,
            tc.tile_pool(name="sup", bufs=1) as p_sup,
            tc.tile_pool(name="slab", bufs=3) as p_slab,
            tc.tile_pool(name="stage", bufs=4) as p_stage,
        ):
            xt_sb = p_pers.tile([128, NI * R], BF16, tag="xt_sb", name="xt_sb")
            w_sb = p_pers.tile([128, NI * OUT_F], BF16, tag="w_sb", name="w_sb")
            for j in range(NI):
                nc.scalar.dma_start(
                    out=xt_sb[:, j * R:(j + 1) * R],
                    in_=xt[j * 128:(j + 1) * 128, :],
                )
                nc.scalar.dma_start(
                    out=w_sb[:, j * OUT_F:(j + 1) * OUT_F],
                    in_=w[j * 128:(j + 1) * 128, :],
                )

            # ---- support_c = x_c @ W (all local), kept bf16 as the PE
            # stationary operand for the main SpMM ----
            supbf = []
            with tc.tile_pool(name="psum_sup", bufs=3, space="PSUM") as pp_sup:
                for s in range(NQ):
                    ps = pp_sup.tile([KT, OUT_F], F32, tag="ps", name=f"ps{s}")
                    for j in range(NI):
                        nc.tensor.matmul(
                            ps[:, :],
                            lhsT=xt_sb[:, j * R + s * KT: j * R + (s + 1) * KT],
                            rhs=w_sb[:, j * OUT_F:(j + 1) * OUT_F],
                            start=(j == 0),
                            stop=(j == NI - 1),
                        )
                    sb = p_sup.tile(
                        [KT, OUT_F], BF16, tag=f"supbf{s}", name=f"supbf{s}"
                    )
                    nc.vector.tensor_copy(sb[:, :], ps[:, :])
                    supbf.append(sb)

            # ---- main SpMM, transposed: for each i-group (2000 rows) and
            # feature chunk j (128), accumulate over the 10 local k-tiles.
            # One compound matmul per (g, j, q) writes 4 PSUM banks
            # (4 x 500 cols) off a single weight load. ----
            HALF = NQ * IG // 2
            with tc.tile_pool(name="psum_main", bufs=1, space="PSUM") as pp_main:
                for g in range(G):
                    slab = p_slab.tile(
                        [KT, NQ * IG], BF16, tag="slab", name=f"slab{g}"
                    )
                    nc.scalar.dma_start(out=slab[:, :], in_=adjp[g, :, :])
                    for j in range(NJ):
                        acc = pp_main.tile(
                            [128, NIC * 512], F32, tag=f"acc{j % 2}",
                            name=f"acc{g}_{j}",
                        )
                        for q in range(NQ):
                            for ic in range(NIC):
                                nc.tensor.matmul(
                                    acc[:, ic * 512: ic * 512 + IC],
                                    lhsT=supbf[q][:, j * FJ:(j + 1) * FJ],
                                    rhs=slab[:, q * IG + ic * IC:
                                             q * IG + (ic + 1) * IC],
                                    start=(q == 0),
                                    stop=(q == NQ - 1),
                                )
                        stage = p_stage.tile(
                            [128, IG], BF16, tag="stage", name=f"stage{g}_{j}"
                        )
                        nc.vector.tensor_copy(
                            stage[:, :].rearrange("p (a b) -> p a b", a=NIC),
                            acc[:, :].rearrange(
                                "p (a b) -> p a b", a=NIC
                            )[:, :, 0:IC],
                        )
                        nc.gpsimd.dma_start(
                            out=rs_in[g][j * FJ:(j + 1) * FJ, :],
                            in_=stage[:, :],
                        )
                    nc.gpsimd.collective_compute(
                        "ReduceScatter",
                        mybir.AluOpType.add,
                        replica_groups=[list(range(C))],
                        ins=[rs_in[g].ap().opt()],
                        outs=[rs_out[g].ap().opt()],
                    )
                    nc.sync.dma_start(out=out[g], in_=rs_out[g][:, :])

    nc.compile()
    return nc


def _shard_inputs(x, adj, t, weight):
    bf16 = ml_dtypes.bfloat16
    t0 = float(np.asarray(t, np.float32).reshape(-1)[0])
    A = np.asarray(adj, np.float32) * t0
    idx = np.arange(N)
    A[idx, idx] += 1.0 - t0
    Ab = A.astype(bf16)                       # [N(i), N(k)] bf16
    x = np.asarray(x, np.float32)
    w_bf = np.asarray(weight, np.float32).astype(bf16)

    in_maps = []
    for c in range(C):
        cols = slice(c * R, (c + 1) * R)
        blk = Ab[:, cols]                     # [10000 i, 1250 k]
        adjpc = np.ascontiguousarray(
            blk.reshape(G, IG, NQ, KT).transpose(0, 3, 2, 1)
        ).reshape(G, KT, NQ * IG)
        xtc = np.ascontiguousarray(x[cols].T).astype(bf16)  # [IN_F, R]
        in_maps.append({"adjp": adjpc, "xt": xtc, "w": w_bf})
    return in_maps


def _assemble(res):
    outT = np.concatenate(
        [np.asarray(res.results[c]["out"]).astype(np.float32)
         .transpose(1, 0, 2).reshape(OC, N)
         for c in range(C)],
        axis=0,
    )                                         # [512, 10000]
    return np.ascontiguousarray(outT.T)       # [10000, 512]


def kernel(x, adj, t, weight):
    if "nc" not in _cached:
        _cached["nc"] = _build()
    nc = _cached["nc"]
    in_maps = _shard_inputs(x, adj, t, weight)
    res = run_bass_kernel_spmd(nc, in_maps, list(range(C)))
    return _assemble(res)


# revision 13
# speedup vs baseline: 1.0896x; 1.0216x over previous
"""AdaptiveDiffusionLayer on 8 TRN2 NeuronCores.

out = (1 - t) * support + t * (adj @ support),  support = x @ weight

Strategy (transposed 1D column-parallel SpMM + feature-sharded ReduceScatter):
  - Fold the identity mix into the matrix on the host: A' = t*adj + (1-t)*I,
    so the device computes a pure A' @ support.
  - Column-shard A' across 8 cores (contraction dim k): core c owns
    A'[:, c*1250:(c+1)*1250]. Shard x by the same k rows so support_c =
    x_c @ W is entirely local.
  - Compute the TRANSPOSED output: out^T[f, i] = sum_k sup[k, f] * A'[i, k].
    The stationary PE operand is a support tile [125k x 128f] (reused for a
    whole 2000-column stream of adj -> ~10x fewer LDWEIGHTS than making adj
    stationary), the moving operand is A'^T streamed straight from HBM.
  - adj is host-packed per core as [5 groups][125 part][10 q][2000 i] so each
    group loads with two 2.5MB dma_starts (125 x 20KB descriptors each), one
    on each HWDGE queue (sync + scalar), spreading across the SDMA engines.
  - Partials [512f x 2000i] per group feed a pipelined ReduceScatter over the
    feature dim (512 = 8 ranks x 64); compute of group g+1 overlaps the
    collective of group g. Each core ends with out^T[64c:64c+64, :]; the host
    concatenates and transposes.
"""

import sys

for _p in ("/opt/trn_rl_repo",):
    if _p not in sys.path:
        sys.path.append(_p)

import numpy as np
import ml_dtypes

from concourse import bass, bacc, mybir, tile
from concourse.bass_utils import run_bass_kernel_spmd

N = 10000
IN_F = 512
OUT_F = 512
C = 8               # cores
R = N // C          # 1250 k rows per core
KT = 125            # k-tile (PE contraction rows)
NQ = R // KT        # 10 k-tiles
G = 5               # i groups
IG = N // G         # 2000 output rows (i) per group
IC = 500            # psum chunk (columns per PSUM bank)
NIC = IG // IC      # 4 chunks per group
FJ = 128            # feature chunk (PE stationary free dim)
NJ = OUT_F // FJ    # 4 feature chunks
NI = IN_F // 128    # 4 support contraction chunks
OC = OUT_F // C     # 64 feature rows per rank after ReduceScatter

BF16 = mybir.dt.bfloat16
F32 = mybir.dt.float32

_cached = {}


def _build():
    nc = bacc.Bacc("TRN2", target_bir_lowering=False, debug=False, num_devices=C)

    adjp = nc.dram_tensor("adjp", [G, KT, NQ * IG], BF16, kind="ExternalInput")
    xt = nc.dram_tensor("xt", [IN_F, R], BF16, kind="ExternalInput")
    w = nc.dram_tensor("w", [IN_F, OUT_F], BF16, kind="ExternalInput")
    out = nc.dram_tensor("out", [G, OC, IG], BF16, kind="ExternalOutput")

    rs_in = [nc.dram_tensor(f"rs_in{g}", [OUT_F, IG], BF16) for g in range(G)]
    rs_out = [nc.dram_tensor(f"rs_out{g}", [OC, IG], BF16) for g in range(G)]

    with tile.TileContext(nc) as tc:
        with (
            tc.tile_pool(name="persist", bufs=1) as p_pers,
            tc.tile_pool(name="sup", bufs=1) as p_sup,
            tc.tile_pool(name="slab", bufs=3) as p_slab,
            tc.tile_pool(name="stage", bufs=4) as p_stage,
        ):
            xt_sb = p_pers.tile([128, NI * R], BF16, tag="xt_sb", name="xt_sb")
            w_sb = p_pers.tile([128, NI * OUT_F], BF16, tag="w_sb", name="w_sb")
            for j in range(NI):
                nc.sync.dma_start(
                    out=xt_sb[:, j * R:(j + 1) * R],
                    in_=xt[j * 128:(j + 1) * 128, :],
                )
                nc.scalar.dma_start(
                    out=w_sb[:, j * OUT_F:(j + 1) * OUT_F],
                    in_=w[j * 128:(j + 1) * 128, :],
                )

            # ---- support_c = x_c @ W (all local), kept bf16 as the PE
            # stationary operand for the main SpMM ----
            supbf = []
            with tc.tile_pool(name="psum_sup", bufs=3, space="PSUM") as pp_sup:
                for s in range(NQ):
                    ps = pp_sup.tile([KT, OUT_F], F32, tag="ps", name=f"ps{s}")
                    for j in range(NI):
                        nc.tensor.matmul(
                            ps[:, :],
                            lhsT=xt_sb[:, j * R + s * KT: j * R + (s + 1) * KT],
                            rhs=w_sb[:, j * OUT_F:(j + 1) * OUT_F],
                            start=(j == 0),
                            stop=(j == NI - 1),
                        )
                    sb = p_sup.tile(
                        [KT, OUT_F], BF16, tag=f"supbf{s}", name=f"supbf{s}"
                    )
                    nc.vector.tensor_copy(sb[:, :], ps[:, :])
                    supbf.append(sb)

            # ---- main SpMM, transposed: for each i-group (2000 rows) and
            # feature chunk j (128), accumulate over the 10 local k-tiles.
            # One compound matmul per (g, j, q) writes 4 PSUM banks
            # (4 x 500 cols) off a single weight load. ----
            HALF = NQ * IG // 2
            with tc.tile_pool(name="psum_main", bufs=1, space="PSUM") as pp_main:
                for g in range(G):
                    slab = p_slab.tile(
                        [KT, NQ * IG], BF16, tag="slab", name=f"slab{g}"
                    )
                    nc.sync.dma_start(
                        out=slab[:, 0:HALF], in_=adjp[g, :, 0:HALF]
                    )
                    nc.scalar.dma_start(
                        out=slab[:, HALF:NQ * IG], in_=adjp[g, :, HALF:NQ * IG]
                    )
                    for j in range(NJ):
                        acc = pp_main.tile(
                            [128, NIC * 512], F32, tag=f"acc{j % 2}",
                            name=f"acc{g}_{j}",
                        )
                        for q in range(NQ):
                            for ic in range(NIC):
                                nc.tensor.matmul(
                                    acc[:, ic * 512: ic * 512 + IC],
                                    lhsT=supbf[q][:, j * FJ:(j + 1) * FJ],
                                    rhs=slab[:, q * IG + ic * IC:
                                             q * IG + (ic + 1) * IC],
                                    start=(q == 0),
                                    stop=(q == NQ - 1),
                                )
                        stage = p_stage.tile(
                            [128, IG], BF16, tag="stage", name=f"stage{g}_{j}"
                        )
                        nc.vector.tensor_copy(
                            stage[:, :].rearrange("p (a b) -> p a b", a=NIC),
                            acc[:, :].rearrange(
                                "p (a b) -> p a b", a=NIC
                            )[:, :, 0:IC],
                        )
                        nc.gpsimd.dma_start(
                            out=rs_in[g][j * FJ:(j + 1) * FJ, :],
                            in_=stage[:, :],
                        )
                    nc.gpsimd.collective_compute(
                        "ReduceScatter",
                        mybir.AluOpType.add,
                        replica_groups=[list(range(C))],
                        ins=[rs_in[g].ap().opt()],
                        outs=[rs_out[g].ap().opt()],
                    )
                # final out copies, after every slab dispatch so they can
                # never head-of-line block the adj stream on the sync queue
                for g in range(G):
                    nc.sync.dma_start(out=out[g], in_=rs_out[g][:, :])

    nc.compile()
    return nc


def _shard_inputs(x, adj, t, weight):
    bf16 = ml_dtypes.bfloat16
    t0 = float(np.asarray(t, np.float32).reshape(-1)[0])
    A = np.asarray(adj, np.float32) * t0
    idx = np.arange(N)
    A[idx, idx] += 1.0 - t0
    Ab = A.astype(bf16)                       # [N(i), N(k)] bf16
    x = np.asarray(x, np.float32)
    w_bf = np.asarray(weight, np.float32).astype(bf16)

    in_maps = []
    for c in range(C):
        cols = slice(c * R, (c + 1) * R)
        blk = Ab[:, cols]                     # [10000 i, 1250 k]
        adjpc = np.ascontiguousarray(
            blk.reshape(G, IG, NQ, KT).transpose(0, 3, 2, 1)
        ).reshape(G, KT, NQ * IG)
        xtc = np.ascontiguousarray(x[cols].T).astype(bf16)  # [IN_F, R]
        in_maps.append({"adjp": adjpc, "xt": xtc, "w": w_bf})
    return in_maps


def _assemble(res):
    outT = np.concatenate(
        [np.asarray(res.results[c]["out"]).astype(np.float32)
         .transpose(1, 0, 2).reshape(OC, N)
         for c in range(C)],
        axis=0,
    )                                         # [512, 10000]
    return np.ascontiguousarray(outT.T)       # [10000, 512]


def kernel(x, adj, t, weight):
    if "nc" not in _cached:
        _cached["nc"] = _build()
    nc = _cached["nc"]
    in_maps = _shard_inputs(x, adj, t, weight)
    res = run_bass_kernel_spmd(nc, in_maps, list(range(C)))
    return _assemble(res)


# revision 17
# speedup vs baseline: 1.2729x; 1.1682x over previous
"""AdaptiveDiffusionLayer on 8 TRN2 NeuronCores.

out = (1 - t) * support + t * (adj @ support),  support = x @ weight

Strategy (transposed 1D column-parallel SpMM + feature-sharded ReduceScatter):
  - Fold the identity mix into the matrix on the host: A' = t*adj + (1-t)*I,
    so the device computes a pure A' @ support.
  - Column-shard A' across 8 cores (contraction dim k): core c owns
    A'[:, c*1250:(c+1)*1250]. Shard x by the same k rows so support_c =
    x_c @ W is entirely local.
  - Compute the TRANSPOSED output: out^T[f, i] = sum_k sup[k, f] * A'[i, k].
    The stationary PE operand is a support tile [125k x 128f] (reused for a
    whole 2000-column stream of adj -> ~10x fewer LDWEIGHTS than making adj
    stationary), the moving operand is A'^T streamed straight from HBM.
  - adj is host-packed per core as [5 groups][125 part][10 q][2000 i] so each
    group loads with two 2.5MB dma_starts (125 x 20KB descriptors each), one
    on each HWDGE queue (sync + scalar), spreading across the SDMA engines.
  - Partials [512f x 2000i] per group feed a pipelined ReduceScatter over the
    feature dim (512 = 8 ranks x 64); compute of group g+1 overlaps the
    collective of group g. Each core ends with out^T[64c:64c+64, :]; the host
    concatenates and transposes.
"""

import sys

for _p in ("/opt/trn_rl_repo",):
    if _p not in sys.path:
        sys.path.append(_p)

import numpy as np
import ml_dtypes

from concourse import bass, bacc, mybir, tile
from concourse.bass_utils import run_bass_kernel_spmd

N = 10000
IN_F = 512
OUT_F = 512
C = 8               # cores
R = N // C          # 1250 k rows per core
KT = 128            # k-tile (PE contraction rows; 128 partitions so the
                    # HWDGE spreads descriptors across all 16 SDMA engines)
NQ = 10             # k-tiles per core (9 full + 1 ragged)
KLAST = R - (NQ - 1) * KT  # 98 rows in the last k-tile
G = 5               # i groups
IG = N // G         # 2000 output rows (i) per group
IC = 500            # psum chunk (columns per PSUM bank)
NIC = IG // IC      # 4 chunks per group
FJ = 128            # feature chunk (PE stationary free dim)
NJ = OUT_F // FJ    # 4 feature chunks
NI = IN_F // 128    # 4 support contraction chunks
OC = OUT_F // C     # 64 feature rows per rank after ReduceScatter

BF16 = mybir.dt.bfloat16
F32 = mybir.dt.float32

_cached = {}


def _build():
    nc = bacc.Bacc("TRN2", target_bir_lowering=False, debug=False, num_devices=C)

    adjp = nc.dram_tensor("adjp", [G, KT, NQ * IG], BF16, kind="ExternalInput")
    xt = nc.dram_tensor("xt", [IN_F, R], BF16, kind="ExternalInput")
    w = nc.dram_tensor("w", [IN_F, OUT_F], BF16, kind="ExternalInput")
    out = nc.dram_tensor("out", [G, OC, IG], BF16, kind="ExternalOutput")

    rs_in = [nc.dram_tensor(f"rs_in{g}", [OUT_F, IG], BF16) for g in range(G)]
    rs_out = [nc.dram_tensor(f"rs_out{g}", [OC, IG], BF16) for g in range(G)]

    with tile.TileContext(nc) as tc:
        with (
            tc.tile_pool(name="persist", bufs=1) as p_pers,
            tc.tile_pool(name="sup", bufs=1) as p_sup,
            tc.tile_pool(name="slab", bufs=3) as p_slab,
            tc.tile_pool(name="stage", bufs=4) as p_stage,
        ):
            xt_sb = p_pers.tile([128, NI * R], BF16, tag="xt_sb", name="xt_sb")
            w_sb = p_pers.tile([128, NI * OUT_F], BF16, tag="w_sb", name="w_sb")
            for j in range(NI):
                nc.sync.dma_start(
                    out=xt_sb[:, j * R:(j + 1) * R],
                    in_=xt[j * 128:(j + 1) * 128, :],
                )
                nc.scalar.dma_start(
                    out=w_sb[:, j * OUT_F:(j + 1) * OUT_F],
                    in_=w[j * 128:(j + 1) * 128, :],
                )

            # ---- support_c = x_c @ W (all local), kept bf16 as the PE
            # stationary operand for the main SpMM ----
            supbf = []
            with tc.tile_pool(name="psum_sup", bufs=3, space="PSUM") as pp_sup:
                for s in range(NQ):
                    rows = KT if s < NQ - 1 else KLAST
                    ps = pp_sup.tile([KT, OUT_F], F32, tag="ps", name=f"ps{s}")
                    for j in range(NI):
                        nc.tensor.matmul(
                            ps[0:rows, :],
                            lhsT=xt_sb[:, j * R + s * KT:
                                       j * R + s * KT + rows],
                            rhs=w_sb[:, j * OUT_F:(j + 1) * OUT_F],
                            start=(j == 0),
                            stop=(j == NI - 1),
                        )
                    sb = p_sup.tile(
                        [KT, OUT_F], BF16, tag=f"supbf{s}", name=f"supbf{s}"
                    )
                    nc.vector.tensor_copy(sb[0:rows, :], ps[0:rows, :])
                    supbf.append(sb)

            # ---- main SpMM, transposed: for each i-group (2000 rows) and
            # feature chunk j (128), accumulate over the 10 local k-tiles.
            # One compound matmul per (g, j, q) writes 4 PSUM banks
            # (4 x 500 cols) off a single weight load. ----
            HALF = NQ * IG // 2
            with tc.tile_pool(name="psum_main", bufs=1, space="PSUM") as pp_main:
                for g in range(G):
                    slab = p_slab.tile(
                        [KT, NQ * IG], BF16, tag="slab", name=f"slab{g}"
                    )
                    nc.sync.dma_start(
                        out=slab[:, 0:HALF], in_=adjp[g, :, 0:HALF]
                    )
                    nc.scalar.dma_start(
                        out=slab[:, HALF:NQ * IG], in_=adjp[g, :, HALF:NQ * IG]
                    )
                    for j in range(NJ):
                        acc = pp_main.tile(
                            [128, NIC * 512], F32, tag=f"acc{j % 2}",
                            name=f"acc{g}_{j}",
                        )
                        for q in range(NQ):
                            rows = KT if q < NQ - 1 else KLAST
                            for ic in range(NIC):
                                nc.tensor.matmul(
                                    acc[:, ic * 512: ic * 512 + IC],
                                    lhsT=supbf[q][0:rows, j * FJ:(j + 1) * FJ],
                                    rhs=slab[0:rows, q * IG + ic * IC:
                                             q * IG + (ic + 1) * IC],
                                    start=(q == 0),
                                    stop=(q == NQ - 1),
                                )
                        stage = p_stage.tile(
                            [128, IG], BF16, tag="stage", name=f"stage{g}_{j}"
                        )
                        nc.vector.tensor_copy(
                            stage[:, :].rearrange("p (a b) -> p a b", a=NIC),
                            acc[:, :].rearrange(
                                "p (a b) -> p a b", a=NIC
                            )[:, :, 0:IC],
                        )
                        nc.gpsimd.dma_start(
                            out=rs_in[g][j * FJ:(j + 1) * FJ, :],
                            in_=stage[:, :],
                        )
                    nc.gpsimd.collective_compute(
                        "ReduceScatter",
                        mybir.AluOpType.add,
                        replica_groups=[list(range(C))],
                        ins=[rs_in[g].ap().opt()],
                        outs=[rs_out[g].ap().opt()],
                    )
                # final out copies, after every slab dispatch so they can
                # never head-of-line block the adj stream on the sync queue
                for g in range(G):
                    nc.sync.dma_start(out=out[g], in_=rs_out[g][:, :])

    nc.compile()
    return nc


def _shard_inputs(x, adj, t, weight):
    bf16 = ml_dtypes.bfloat16
    t0 = float(np.asarray(t, np.float32).reshape(-1)[0])
    A = np.asarray(adj, np.float32) * t0
    idx = np.arange(N)
    A[idx, idx] += 1.0 - t0
    Ab = A.astype(bf16)                       # [N(i), N(k)] bf16
    x = np.asarray(x, np.float32)
    w_bf = np.asarray(weight, np.float32).astype(bf16)

    in_maps = []
    for c in range(C):
        cols = slice(c * R, (c + 1) * R)
        blk = np.zeros((N, NQ * KT), dtype=bf16)  # k padded 1250 -> 1280
        blk[:, :R] = Ab[:, cols]              # [10000 i, 1280 k]
        adjpc = np.ascontiguousarray(
            blk.reshape(G, IG, NQ, KT).transpose(0, 3, 2, 1)
        ).reshape(G, KT, NQ * IG)
        xtc = np.ascontiguousarray(x[cols].T).astype(bf16)  # [IN_F, R]
        in_maps.append({"adjp": adjpc, "xt": xtc, "w": w_bf})
    return in_maps


def _assemble(res):
    outT = np.concatenate(
        [np.asarray(res.results[c]["out"]).astype(np.float32)
         .transpose(1, 0, 2).reshape(OC, N)
         for c in range(C)],
        axis=0,
    )                                         # [512, 10000]
    return np.ascontiguousarray(outT.T)       # [10000, 512]


def kernel(x, adj, t, weight):
    if "nc" not in _cached:
        _cached["nc"] = _build()
    nc = _cached["nc"]
    in_maps = _shard_inputs(x, adj, t, weight)
    res = run_bass_kernel_spmd(nc, in_maps, list(range(C)))
    return _assemble(res)


# revision 18
# speedup vs baseline: 1.2780x; 1.0040x over previous
"""AdaptiveDiffusionLayer on 8 TRN2 NeuronCores.

out = (1 - t) * support + t * (adj @ support),  support = x @ weight

Strategy (transposed 1D column-parallel SpMM + feature-sharded ReduceScatter):
  - Fold the identity mix into the matrix on the host: A' = t*adj + (1-t)*I,
    so the device computes a pure A' @ support.
  - Column-shard A' across 8 cores (contraction dim k): core c owns
    A'[:, c*1250:(c+1)*1250]. Shard x by the same k rows so support_c =
    x_c @ W is entirely local.
  - Compute the TRANSPOSED output: out^T[f, i] = sum_k sup[k, f] * A'[i, k].
    The stationary PE operand is a support tile [128k x 128f] (reused for a
    whole i-group stream of adj -> few LDWEIGHTS; redundant consecutive
    LDWEIGHTS are deleted post-schedule), the moving operand is A'^T
    streamed straight from HBM.
  - k is tiled 9x128 + 98 (128 SBUF partitions so the HWDGE spreads each
    DMA's descriptors across all 16 SDMA engines; 125-partition transfers
    land on only 5 engines). adj is host-packed per core as
    [128 part][group][10 q][IG i] and each group loads with two ~2.5MB
    dma_starts (20KB descriptors), one per HWDGE queue (sync + scalar).
  - Partials [512f x IG] per i-group feed a pipelined ReduceScatter over
    the feature dim (512 = 8 ranks x 64). Group sizes taper
    (2000x4, 1500, 500) so the final collective (the unhidable tail) is
    small. Final rs_out -> out copies ride the sync queue after all slab
    dispatches (no head-of-line blocking of the adj stream).
"""

import sys

for _p in ("/opt/trn_rl_repo",):
    if _p not in sys.path:
        sys.path.append(_p)

import numpy as np
import ml_dtypes

from concourse import bass, bacc, mybir, tile
from concourse.bass_utils import run_bass_kernel_spmd

N = 10000
IN_F = 512
OUT_F = 512
C = 8               # cores
R = N // C          # 1250 k rows per core
KT = 128            # k-tile (PE contraction rows / SBUF partitions)
NQ = 10             # k-tiles per core (9 full + 1 ragged)
KLAST = R - (NQ - 1) * KT  # 98 rows in the last k-tile
GS = [2000, 2000, 2000, 2000, 1500, 500]   # i rows per group (tapered)
OFF = [sum(GS[:g]) for g in range(len(GS))]
G = len(GS)
IC = 500            # psum chunk (columns per PSUM bank)
NICS = [gs // IC for gs in GS]
FJ = 128            # feature chunk (PE stationary free dim)
NJ = OUT_F // FJ    # 4 feature chunks
NI = IN_F // 128    # 4 support contraction chunks
OC = OUT_F // C     # 64 feature rows per rank after ReduceScatter
IGMAX = max(GS)

BF16 = mybir.dt.bfloat16
F32 = mybir.dt.float32

_cached = {}


def _dedup_ldweights(nc):
    """Delete InstLdweights whose weights AP is identical to the previous
    weight load on the PE queue (the array contents are unchanged between
    them; matmuls here are non-self-loading)."""
    deleted = set()
    for blk in nc.main_func.blocks:
        prev = None
        idxs = []
        for i, inst in enumerate(blk.instructions):
            tn = type(inst).__name__
            if tn == "InstLdweights":
                key = str(inst.ins[0])
                if key == prev:
                    idxs.append(i)
                    deleted.add(inst.name)
                else:
                    prev = key
            elif tn == "InstMatmult":
                if inst.ldweights:
                    prev = None
        for i in reversed(idxs):
            del blk.instructions[i]
    if not deleted:
        return
    # safety: no surviving instruction may reference a deleted one
    for blk in nc.main_func.blocks:
        for inst in blk.instructions:
            for d in inst.sync_dependency_names():
                assert d not in deleted, f"{inst.name} depends on deleted {d}"
            for d in inst.nosync_dependency_names():
                assert d not in deleted, f"{inst.name} depends on deleted {d}"


def _build():
    nc = bacc.Bacc("TRN2", target_bir_lowering=False, debug=False, num_devices=C)

    adjp = nc.dram_tensor("adjp", [KT, NQ * N], BF16, kind="ExternalInput")
    xt = nc.dram_tensor("xt", [IN_F, R], BF16, kind="ExternalInput")
    w = nc.dram_tensor("w", [IN_F, OUT_F], BF16, kind="ExternalInput")
    out = nc.dram_tensor("out", [OC, N], BF16, kind="ExternalOutput")

    rs_in = [nc.dram_tensor(f"rs_in{g}", [OUT_F, GS[g]], BF16)
             for g in range(G)]
    rs_out = [nc.dram_tensor(f"rs_out{g}", [OC, GS[g]], BF16)
              for g in range(G)]

    with tile.TileContext(nc) as tc:
        with (
            tc.tile_pool(name="persist", bufs=1) as p_pers,
            tc.tile_pool(name="sup", bufs=1) as p_sup,
            tc.tile_pool(name="slab", bufs=3) as p_slab,
            tc.tile_pool(name="stage", bufs=4) as p_stage,
        ):
            xt_sb = p_pers.tile([128, NI * R], BF16, tag="xt_sb", name="xt_sb")
            w_sb = p_pers.tile([128, NI * OUT_F], BF16, tag="w_sb", name="w_sb")
            for j in range(NI):
                nc.sync.dma_start(
                    out=xt_sb[:, j * R:(j + 1) * R],
                    in_=xt[j * 128:(j + 1) * 128, :],
                )
                nc.scalar.dma_start(
                    out=w_sb[:, j * OUT_F:(j + 1) * OUT_F],
                    in_=w[j * 128:(j + 1) * 128, :],
                )

            # ---- support_c = x_c @ W (all local), kept bf16 as the PE
            # stationary operand for the main SpMM ----
            supbf = []
            with tc.tile_pool(name="psum_sup", bufs=3, space="PSUM") as pp_sup:
                for s in range(NQ):
                    rows = KT if s < NQ - 1 else KLAST
                    ps = pp_sup.tile([KT, OUT_F], F32, tag="ps", name=f"ps{s}")
                    for j in range(NI):
                        nc.tensor.matmul(
                            ps[0:rows, :],
                            lhsT=xt_sb[:, j * R + s * KT:
                                       j * R + s * KT + rows],
                            rhs=w_sb[:, j * OUT_F:(j + 1) * OUT_F],
                            start=(j == 0),
                            stop=(j == NI - 1),
                        )
                    sb = p_sup.tile(
                        [KT, OUT_F], BF16, tag=f"supbf{s}", name=f"supbf{s}"
                    )
                    nc.vector.tensor_copy(sb[0:rows, :], ps[0:rows, :])
                    supbf.append(sb)

            # ---- main SpMM, transposed: per i-group and feature chunk j,
            # accumulate over the 10 local k-tiles. ----
            with tc.tile_pool(name="psum_main", bufs=1, space="PSUM") as pp_main:
                for g in range(G):
                    gs, nic = GS[g], NICS[g]
                    base = NQ * OFF[g]
                    half = NQ * gs // 2
                    slab = p_slab.tile(
                        [KT, NQ * IGMAX], BF16, tag="slab", name=f"slab{g}"
                    )
                    nc.sync.dma_start(
                        out=slab[:, 0:half],
                        in_=adjp[:, base:base + half],
                    )
                    nc.scalar.dma_start(
                        out=slab[:, half:NQ * gs],
                        in_=adjp[:, base + half:base + NQ * gs],
                    )
                    for j in range(NJ):
                        acc = pp_main.tile(
                            [128, 4 * 512], F32, tag=f"acc{j % 2}",
                            name=f"acc{g}_{j}",
                        )
                        for q in range(NQ):
                            rows = KT if q < NQ - 1 else KLAST
                            for ic in range(nic):
                                nc.tensor.matmul(
                                    acc[:, ic * 512: ic * 512 + IC],
                                    lhsT=supbf[q][0:rows, j * FJ:(j + 1) * FJ],
                                    rhs=slab[0:rows, q * gs + ic * IC:
                                             q * gs + (ic + 1) * IC],
                                    start=(q == 0),
                                    stop=(q == NQ - 1),
                                )
                        stage = p_stage.tile(
                            [128, IGMAX], BF16, tag="stage", name=f"stage{g}_{j}"
                        )
                        nc.vector.tensor_copy(
                            stage[:, 0:gs].rearrange("p (a b) -> p a b", a=nic),
                            acc[:, 0:nic * 512].rearrange(
                                "p (a b) -> p a b", a=nic
                            )[:, :, 0:IC],
                        )
                        nc.gpsimd.dma_start(
                            out=rs_in[g][j * FJ:(j + 1) * FJ, :],
                            in_=stage[:, 0:gs],
                        )
                    nc.gpsimd.collective_compute(
                        "ReduceScatter",
                        mybir.AluOpType.add,
                        replica_groups=[list(range(C))],
                        ins=[rs_in[g].ap().opt()],
                        outs=[rs_out[g].ap().opt()],
                    )
                # final out copies, after every slab dispatch so they can
                # never head-of-line block the adj stream on the sync queue
                for g in range(G):
                    nc.sync.dma_start(
                        out=out[:, OFF[g]:OFF[g] + GS[g]],
                        in_=rs_out[g][:, :],
                    )

    _dedup_ldweights(nc)
    nc.compile()
    return nc


def _shard_inputs(x, adj, t, weight):
    bf16 = ml_dtypes.bfloat16
    t0 = float(np.asarray(t, np.float32).reshape(-1)[0])
    A = np.asarray(adj, np.float32) * t0
    idx = np.arange(N)
    A[idx, idx] += 1.0 - t0
    Ab = A.astype(bf16)                       # [N(i), N(k)] bf16
    x = np.asarray(x, np.float32)
    w_bf = np.asarray(weight, np.float32).astype(bf16)

    in_maps = []
    for c in range(C):
        cols = slice(c * R, (c + 1) * R)
        blk = np.zeros((N, NQ * KT), dtype=bf16)  # k padded 1250 -> 1280
        blk[:, :R] = Ab[:, cols]              # [10000 i, 1280 k]
        parts = []
        for g in range(G):
            bg = blk[OFF[g]:OFF[g] + GS[g]]   # [gs, 1280]
            parts.append(
                bg.reshape(GS[g], NQ, KT).transpose(2, 1, 0)
                .reshape(KT, NQ * GS[g])
            )
        adjpc = np.ascontiguousarray(np.concatenate(parts, axis=1))
        xtc = np.ascontiguousarray(x[cols].T).astype(bf16)  # [IN_F, R]
        in_maps.append({"adjp": adjpc, "xt": xtc, "w": w_bf})
    return in_maps


def _assemble(res):
    outT = np.concatenate(
        [np.asarray(res.results[c]["out"]).astype(np.float32)
         for c in range(C)],
        axis=0,
    )                                         # [512, 10000]
    return np.ascontiguousarray(outT.T)       # [10000, 512]


def kernel(x, adj, t, weight):
    if "nc" not in _cached:
        _cached["nc"] = _build()
    nc = _cached["nc"]
    in_maps = _shard_inputs(x, adj, t, weight)
    res = run_bass_kernel_spmd(nc, in_maps, list(range(C)))
    return _assemble(res)


# revision 19
# speedup vs baseline: 1.2877x; 1.0076x over previous
"""AdaptiveDiffusionLayer on 8 TRN2 NeuronCores.

out = (1 - t) * support + t * (adj @ support),  support = x @ weight

Strategy (transposed 1D column-parallel SpMM + feature-sharded ReduceScatter):
  - Fold the identity mix into the matrix on the host: A' = t*adj + (1-t)*I,
    so the device computes a pure A' @ support.
  - Column-shard A' across 8 cores (contraction dim k): core c owns
    A'[:, c*1250:(c+1)*1250]. Shard x by the same k rows so support_c =
    x_c @ W is entirely local.
  - Compute the TRANSPOSED output: out^T[f, i] = sum_k sup[k, f] * A'[i, k].
    The stationary PE operand is a support tile [128k x 128f] (reused for a
    whole i-group stream of adj -> few LDWEIGHTS; redundant consecutive
    LDWEIGHTS are deleted post-schedule), the moving operand is A'^T
    streamed straight from HBM.
  - k is tiled 9x128 + 98 (128 SBUF partitions so the HWDGE spreads each
    DMA's descriptors across all 16 SDMA engines; 125-partition transfers
    land on only 5 engines). adj is host-packed per core as
    [128 part][group][10 q][IG i] and each group loads with two ~2.5MB
    dma_starts (20KB descriptors), one per HWDGE queue (sync + scalar).
  - Partials [512f x IG] per i-group feed a pipelined ReduceScatter over
    the feature dim (512 = 8 ranks x 64). Group sizes taper
    (2000x4, 1500, 500) so the final collective (the unhidable tail) is
    small. Final rs_out -> out copies ride the sync queue after all slab
    dispatches (no head-of-line blocking of the adj stream).
"""

import sys

for _p in ("/opt/trn_rl_repo",):
    if _p not in sys.path:
        sys.path.append(_p)

import numpy as np
import ml_dtypes

from concourse import bass, bacc, mybir, tile
from concourse.bass_utils import run_bass_kernel_spmd

N = 10000
IN_F = 512
OUT_F = 512
C = 8               # cores
R = N // C          # 1250 k rows per core
KT = 128            # k-tile (PE contraction rows / SBUF partitions)
NQ = 10             # k-tiles per core (9 full + 1 ragged)
KLAST = R - (NQ - 1) * KT  # 98 rows in the last k-tile
GS = [1000, 2000, 2000, 2000, 2000, 500, 500]  # i rows per group: small
# first group so the serialized ReduceScatter chain starts early, small
# last groups so the unhidable final collectives are cheap
OFF = [sum(GS[:g]) for g in range(len(GS))]
G = len(GS)
IC = 500            # psum chunk (columns per PSUM bank)
NICS = [gs // IC for gs in GS]
FJ = 128            # feature chunk (PE stationary free dim)
NJ = OUT_F // FJ    # 4 feature chunks
NI = IN_F // 128    # 4 support contraction chunks
OC = OUT_F // C     # 64 feature rows per rank after ReduceScatter
IGMAX = max(GS)

BF16 = mybir.dt.bfloat16
F32 = mybir.dt.float32

_cached = {}


def _dedup_ldweights(nc):
    """Delete InstLdweights whose weights AP is identical to the previous
    weight load on the PE queue (the array contents are unchanged between
    them; matmuls here are non-self-loading)."""
    deleted = set()
    for blk in nc.main_func.blocks:
        prev = None
        idxs = []
        for i, inst in enumerate(blk.instructions):
            tn = type(inst).__name__
            if tn == "InstLdweights":
                key = str(inst.ins[0])
                if key == prev:
                    idxs.append(i)
                    deleted.add(inst.name)
                else:
                    prev = key
            elif tn == "InstMatmult":
                if inst.ldweights:
                    prev = None
        for i in reversed(idxs):
            del blk.instructions[i]
    if not deleted:
        return
    # safety: no surviving instruction may reference a deleted one
    for blk in nc.main_func.blocks:
        for inst in blk.instructions:
            for d in inst.sync_dependency_names():
                assert d not in deleted, f"{inst.name} depends on deleted {d}"
            for d in inst.nosync_dependency_names():
                assert d not in deleted, f"{inst.name} depends on deleted {d}"


def _build():
    nc = bacc.Bacc("TRN2", target_bir_lowering=False, debug=False, num_devices=C)

    adjp = nc.dram_tensor("adjp", [KT, NQ * N], BF16, kind="ExternalInput")
    xt = nc.dram_tensor("xt", [IN_F, R], BF16, kind="ExternalInput")
    w = nc.dram_tensor("w", [IN_F, OUT_F], BF16, kind="ExternalInput")
    out = nc.dram_tensor("out", [OC, N], BF16, kind="ExternalOutput")

    rs_in = [nc.dram_tensor(f"rs_in{g}", [OUT_F, GS[g]], BF16)
             for g in range(G)]
    rs_out = [nc.dram_tensor(f"rs_out{g}", [OC, GS[g]], BF16)
              for g in range(G)]

    with tile.TileContext(nc) as tc:
        with (
            tc.tile_pool(name="persist", bufs=1) as p_pers,
            tc.tile_pool(name="sup", bufs=1) as p_sup,
            tc.tile_pool(name="slab", bufs=3) as p_slab,
            tc.tile_pool(name="stage", bufs=4) as p_stage,
        ):
            xt_sb = p_pers.tile([128, NI * R], BF16, tag="xt_sb", name="xt_sb")
            w_sb = p_pers.tile([128, NI * OUT_F], BF16, tag="w_sb", name="w_sb")
            for j in range(NI):
                nc.sync.dma_start(
                    out=xt_sb[:, j * R:(j + 1) * R],
                    in_=xt[j * 128:(j + 1) * 128, :],
                )
                nc.scalar.dma_start(
                    out=w_sb[:, j * OUT_F:(j + 1) * OUT_F],
                    in_=w[j * 128:(j + 1) * 128, :],
                )

            # ---- support_c = x_c @ W (all local), kept bf16 as the PE
            # stationary operand for the main SpMM ----
            supbf = []
            with tc.tile_pool(name="psum_sup", bufs=3, space="PSUM") as pp_sup:
                for s in range(NQ):
                    rows = KT if s < NQ - 1 else KLAST
                    ps = pp_sup.tile([KT, OUT_F], F32, tag="ps", name=f"ps{s}")
                    for j in range(NI):
                        nc.tensor.matmul(
                            ps[0:rows, :],
                            lhsT=xt_sb[:, j * R + s * KT:
                                       j * R + s * KT + rows],
                            rhs=w_sb[:, j * OUT_F:(j + 1) * OUT_F],
                            start=(j == 0),
                            stop=(j == NI - 1),
                        )
                    sb = p_sup.tile(
                        [KT, OUT_F], BF16, tag=f"supbf{s}", name=f"supbf{s}"
                    )
                    nc.vector.tensor_copy(sb[0:rows, :], ps[0:rows, :])
                    supbf.append(sb)

            # ---- main SpMM, transposed: per i-group and feature chunk j,
            # accumulate over the 10 local k-tiles. ----
            with tc.tile_pool(name="psum_main", bufs=1, space="PSUM") as pp_main:
                for g in range(G):
                    gs, nic = GS[g], NICS[g]
                    base = NQ * OFF[g]
                    half = NQ * gs // 2
                    slab = p_slab.tile(
                        [KT, NQ * IGMAX], BF16, tag="slab", name=f"slab{g}"
                    )
                    nc.sync.dma_start(
                        out=slab[:, 0:half],
                        in_=adjp[:, base:base + half],
                    )
                    nc.scalar.dma_start(
                        out=slab[:, half:NQ * gs],
                        in_=adjp[:, base + half:base + NQ * gs],
                    )
                    for j in range(NJ):
                        acc = pp_main.tile(
                            [128, 4 * 512], F32, tag=f"acc{j % 2}",
                            name=f"acc{g}_{j}",
                        )
                        for q in range(NQ):
                            rows = KT if q < NQ - 1 else KLAST
                            for ic in range(nic):
                                nc.tensor.matmul(
                                    acc[:, ic * 512: ic * 512 + IC],
                                    lhsT=supbf[q][0:rows, j * FJ:(j + 1) * FJ],
                                    rhs=slab[0:rows, q * gs + ic * IC:
                                             q * gs + (ic + 1) * IC],
                                    start=(q == 0),
                                    stop=(q == NQ - 1),
                                )
                        stage = p_stage.tile(
                            [128, IGMAX], BF16, tag="stage", name=f"stage{g}_{j}"
                        )
                        nc.vector.tensor_copy(
                            stage[:, 0:gs].rearrange("p (a b) -> p a b", a=nic),
                            acc[:, 0:nic * 512].rearrange(
                                "p (a b) -> p a b", a=nic
                            )[:, :, 0:IC],
                        )
                        nc.gpsimd.dma_start(
                            out=rs_in[g][j * FJ:(j + 1) * FJ, :],
                            in_=stage[:, 0:gs],
                        )
                    nc.gpsimd.collective_compute(
                        "ReduceScatter",
                        mybir.AluOpType.add,
                        replica_groups=[list(range(C))],
                        ins=[rs_in[g].ap().opt()],
                        outs=[rs_out[g].ap().opt()],
                    )
                # final out copies, after every slab dispatch so they can
                # never head-of-line block the adj stream on the sync queue
                for g in range(G):
                    nc.sync.dma_start(
                        out=out[:, OFF[g]:OFF[g] + GS[g]],
                        in_=rs_out[g][:, :],
                    )

    _dedup_ldweights(nc)
    nc.compile()
    return nc


def _shard_inputs(x, adj, t, weight):
    bf16 = ml_dtypes.bfloat16
    t0 = float(np.asarray(t, np.float32).reshape(-1)[0])
    A = np.asarray(adj, np.float32) * t0
    idx = np.arange(N)
    A[idx, idx] += 1.0 - t0
    Ab = A.astype(bf16)                       # [N(i), N(k)] bf16
    x = np.asarray(x, np.float32)
    w_bf = np.asarray(weight, np.float32).astype(bf16)

    in_maps = []
    for c in range(C):
        cols = slice(c * R, (c + 1) * R)
        blk = np.zeros((N, NQ * KT), dtype=bf16)  # k padded 1250 -> 1280
        blk[:, :R] = Ab[:, cols]              # [10000 i, 1280 k]
        parts = []
        for g in range(G):
            bg = blk[OFF[g]:OFF[g] + GS[g]]   # [gs, 1280]
            parts.append(
                bg.reshape(GS[g], NQ, KT).transpose(2, 1, 0)
                .reshape(KT, NQ * GS[g])
            )
        adjpc = np.ascontiguousarray(np.concatenate(parts, axis=1))
        xtc = np.ascontiguousarray(x[cols].T).astype(bf16)  # [IN_F, R]
        in_maps.append({"adjp": adjpc, "xt": xtc, "w": w_bf})
    return in_maps


def _assemble(res):
    outT = np.concatenate(
        [np.asarray(res.results[c]["out"]).astype(np.float32)
         for c in range(C)],
        axis=0,
    )                                         # [512, 10000]
    return np.ascontiguousarray(outT.T)       # [10000, 512]


def kernel(x, adj, t, weight):
    if "nc" not in _cached:
        _cached["nc"] = _build()
    nc = _cached["nc"]
    in_maps = _shard_inputs(x, adj, t, weight)
    res = run_bass_kernel_spmd(nc, in_maps, list(range(C)))
    return _assemble(res)


# revision 20
# speedup vs baseline: 1.3208x; 1.0257x over previous
"""AdaptiveDiffusionLayer on 8 TRN2 NeuronCores.

out = (1 - t) * support + t * (adj @ support),  support = x @ weight

Strategy (transposed 1D column-parallel SpMM + feature-sharded ReduceScatter):
  - Fold the identity mix into the matrix on the host: A' = t*adj + (1-t)*I,
    so the device computes a pure A' @ support.
  - Column-shard A' across 8 cores (contraction dim k): core c owns
    A'[:, c*1250:(c+1)*1250]. Shard x by the same k rows so support_c =
    x_c @ W is entirely local.
  - Compute the TRANSPOSED output: out^T[f, i] = sum_k sup[k, f] * A'[i, k].
    The stationary PE operand is a support tile [128k x 128f] (reused for a
    whole i-group stream of adj -> few LDWEIGHTS; redundant consecutive
    LDWEIGHTS are deleted post-schedule), the moving operand is A'^T
    streamed straight from HBM.
  - k is tiled 9x128 + 98 (128 SBUF partitions so the HWDGE spreads each
    DMA's descriptors across all 16 SDMA engines; 125-partition transfers
    land on only 5 engines). adj is host-packed per core as
    [128 part][group][10 q][IG i] and each group loads with two ~2.5MB
    dma_starts (20KB descriptors), one per HWDGE queue (sync + scalar).
  - Partials [512f x IG] per i-group feed a pipelined ReduceScatter over
    the feature dim (512 = 8 ranks x 64). Group sizes taper
    (2000x4, 1500, 500) so the final collective (the unhidable tail) is
    small. Final rs_out -> out copies ride the sync queue after all slab
    dispatches (no head-of-line blocking of the adj stream).
"""

import sys

for _p in ("/opt/trn_rl_repo",):
    if _p not in sys.path:
        sys.path.append(_p)

import numpy as np
import ml_dtypes

from concourse import bass, bacc, mybir, tile
from concourse.bass_utils import run_bass_kernel_spmd

N = 10000
IN_F = 512
OUT_F = 512
C = 8               # cores
# 2D sharding: k 4-way x i 2-way. Core c owns k-quarter c%4 and i-half c//4.
# ReduceScatter runs over 4-rank groups {0-3} and {4-7} (half the bytes and
# half the summands of the 1D 8-rank version; the two groups' collectives
# run concurrently on their own cores).
KW = 4              # k shards
IW = 2              # i shards
R = N // KW         # 2500 k rows per core
NIH = N // IW       # 5000 i rows per core
KT = 128            # k-tile (PE contraction rows / SBUF partitions)
NQ = 20             # k-tiles per core (19 full + 1 ragged)
KLAST = R - (NQ - 1) * KT  # 68 rows in the last k-tile
GS = [500, 1000, 1000, 1000, 1000, 500]  # i rows per group: small first
# group so the serialized ReduceScatter chain starts early, small last
# group so the unhidable final collective is cheap
OFF = [sum(GS[:g]) for g in range(len(GS))]
G = len(GS)
IC = 500            # psum chunk (columns per PSUM bank)
NICS = [gs // IC for gs in GS]
FJ = 128            # feature chunk (PE stationary free dim)
NJ = OUT_F // FJ    # 4 feature chunks
NI = IN_F // 128    # 4 support contraction chunks
OC = OUT_F // KW    # 128 feature rows per rank after 4-rank ReduceScatter
IGMAX = max(GS)

BF16 = mybir.dt.bfloat16
F32 = mybir.dt.float32

_cached = {}


def _dedup_ldweights(nc):
    """Delete InstLdweights whose weights AP is identical to the previous
    weight load on the PE queue (the array contents are unchanged between
    them; matmuls here are non-self-loading)."""
    deleted = set()
    for blk in nc.main_func.blocks:
        prev = None
        idxs = []
        for i, inst in enumerate(blk.instructions):
            tn = type(inst).__name__
            if tn == "InstLdweights":
                key = str(inst.ins[0])
                if key == prev:
                    idxs.append(i)
                    deleted.add(inst.name)
                else:
                    prev = key
            elif tn == "InstMatmult":
                if inst.ldweights:
                    prev = None
        for i in reversed(idxs):
            del blk.instructions[i]
    if not deleted:
        return
    # safety: no surviving instruction may reference a deleted one
    for blk in nc.main_func.blocks:
        for inst in blk.instructions:
            for d in inst.sync_dependency_names():
                assert d not in deleted, f"{inst.name} depends on deleted {d}"
            for d in inst.nosync_dependency_names():
                assert d not in deleted, f"{inst.name} depends on deleted {d}"


def _build():
    nc = bacc.Bacc("TRN2", target_bir_lowering=False, debug=False, num_devices=C)

    adjp = nc.dram_tensor("adjp", [KT, NQ * NIH], BF16, kind="ExternalInput")
    xt = nc.dram_tensor("xt", [IN_F, R], BF16, kind="ExternalInput")
    w = nc.dram_tensor("w", [IN_F, OUT_F], BF16, kind="ExternalInput")
    out = nc.dram_tensor("out", [OC, NIH], BF16, kind="ExternalOutput")

    rs_in = [nc.dram_tensor(f"rs_in{g}", [OUT_F, GS[g]], BF16)
             for g in range(G)]
    rs_out = [nc.dram_tensor(f"rs_out{g}", [OC, GS[g]], BF16)
              for g in range(G)]

    with tile.TileContext(nc) as tc:
        with (
            tc.tile_pool(name="persist", bufs=1) as p_pers,
            tc.tile_pool(name="sup", bufs=1) as p_sup,
            tc.tile_pool(name="slab", bufs=3) as p_slab,
            tc.tile_pool(name="stage", bufs=4) as p_stage,
        ):
            xt_sb = p_pers.tile([128, NI * R], BF16, tag="xt_sb", name="xt_sb")
            w_sb = p_pers.tile([128, NI * OUT_F], BF16, tag="w_sb", name="w_sb")
            for j in range(NI):
                nc.sync.dma_start(
                    out=xt_sb[:, j * R:(j + 1) * R],
                    in_=xt[j * 128:(j + 1) * 128, :],
                )
                nc.scalar.dma_start(
                    out=w_sb[:, j * OUT_F:(j + 1) * OUT_F],
                    in_=w[j * 128:(j + 1) * 128, :],
                )

            # ---- support_c = x_c @ W (all local), kept bf16 as the PE
            # stationary operand for the main SpMM ----
            supbf = []
            with tc.tile_pool(name="psum_sup", bufs=3, space="PSUM") as pp_sup:
                for s in range(NQ):
                    rows = KT if s < NQ - 1 else KLAST
                    ps = pp_sup.tile([KT, OUT_F], F32, tag="ps", name=f"ps{s}")
                    for j in range(NI):
                        nc.tensor.matmul(
                            ps[0:rows, :],
                            lhsT=xt_sb[:, j * R + s * KT:
                                       j * R + s * KT + rows],
                            rhs=w_sb[:, j * OUT_F:(j + 1) * OUT_F],
                            start=(j == 0),
                            stop=(j == NI - 1),
                        )
                    sb = p_sup.tile(
                        [KT, OUT_F], BF16, tag=f"supbf{s}", name=f"supbf{s}"
                    )
                    nc.vector.tensor_copy(sb[0:rows, :], ps[0:rows, :])
                    supbf.append(sb)

            # ---- main SpMM, transposed: per i-group and feature chunk j,
            # accumulate over the 10 local k-tiles. ----
            with tc.tile_pool(name="psum_main", bufs=1, space="PSUM") as pp_main:
                for g in range(G):
                    gs, nic = GS[g], NICS[g]
                    base = NQ * OFF[g]
                    half = NQ * gs // 2
                    slab = p_slab.tile(
                        [KT, NQ * IGMAX], BF16, tag="slab", name=f"slab{g}"
                    )
                    nc.sync.dma_start(
                        out=slab[:, 0:half],
                        in_=adjp[:, base:base + half],
                    )
                    nc.scalar.dma_start(
                        out=slab[:, half:NQ * gs],
                        in_=adjp[:, base + half:base + NQ * gs],
                    )
                    for j in range(NJ):
                        acc = pp_main.tile(
                            [128, 4 * 512], F32, tag=f"acc{j % 2}",
                            name=f"acc{g}_{j}",
                        )
                        for q in range(NQ):
                            rows = KT if q < NQ - 1 else KLAST
                            for ic in range(nic):
                                nc.tensor.matmul(
                                    acc[:, ic * 512: ic * 512 + IC],
                                    lhsT=supbf[q][0:rows, j * FJ:(j + 1) * FJ],
                                    rhs=slab[0:rows, q * gs + ic * IC:
                                             q * gs + (ic + 1) * IC],
                                    start=(q == 0),
                                    stop=(q == NQ - 1),
                                )
                        stage = p_stage.tile(
                            [128, IGMAX], BF16, tag="stage", name=f"stage{g}_{j}"
                        )
                        nc.vector.tensor_copy(
                            stage[:, 0:gs].rearrange("p (a b) -> p a b", a=nic),
                            acc[:, 0:nic * 512].rearrange(
                                "p (a b) -> p a b", a=nic
                            )[:, :, 0:IC],
                        )
                        nc.gpsimd.dma_start(
                            out=rs_in[g][j * FJ:(j + 1) * FJ, :],
                            in_=stage[:, 0:gs],
                        )
                    nc.gpsimd.collective_compute(
                        "ReduceScatter",
                        mybir.AluOpType.add,
                        replica_groups=[[0, 1, 2, 3], [4, 5, 6, 7]],
                        ins=[rs_in[g].ap().opt()],
                        outs=[rs_out[g].ap().opt()],
                    )
                # final out copies, after every slab dispatch so they can
                # never head-of-line block the adj stream on the sync queue
                for g in range(G):
                    nc.sync.dma_start(
                        out=out[:, OFF[g]:OFF[g] + GS[g]],
                        in_=rs_out[g][:, :],
                    )

    _dedup_ldweights(nc)
    nc.compile()
    return nc


def _shard_inputs(x, adj, t, weight):
    bf16 = ml_dtypes.bfloat16
    t0 = float(np.asarray(t, np.float32).reshape(-1)[0])
    A = np.asarray(adj, np.float32) * t0
    idx = np.arange(N)
    A[idx, idx] += 1.0 - t0
    Ab = A.astype(bf16)                       # [N(i), N(k)] bf16
    x = np.asarray(x, np.float32)
    w_bf = np.asarray(weight, np.float32).astype(bf16)

    in_maps = []
    for c in range(C):
        kq, ih = c % KW, c // KW
        cols = slice(kq * R, (kq + 1) * R)
        rows = slice(ih * NIH, (ih + 1) * NIH)
        blk = np.zeros((NIH, NQ * KT), dtype=bf16)  # k padded 2500 -> 2560
        blk[:, :R] = Ab[rows, cols]           # [5000 i, 2560 k]
        parts = []
        for g in range(G):
            bg = blk[OFF[g]:OFF[g] + GS[g]]   # [gs, 2560]
            parts.append(
                bg.reshape(GS[g], NQ, KT).transpose(2, 1, 0)
                .reshape(KT, NQ * GS[g])
            )
        adjpc = np.ascontiguousarray(np.concatenate(parts, axis=1))
        xtc = np.ascontiguousarray(x[cols].T).astype(bf16)  # [IN_F, R]
        in_maps.append({"adjp": adjpc, "xt": xtc, "w": w_bf})
    return in_maps


def _assemble(res):
    outT = np.empty((OUT_F, N), np.float32)
    for c in range(C):
        kq, ih = c % KW, c // KW
        outT[kq * OC:(kq + 1) * OC, ih * NIH:(ih + 1) * NIH] = \
            np.asarray(res.results[c]["out"]).astype(np.float32)
    return np.ascontiguousarray(outT.T)       # [10000, 512]


def kernel(x, adj, t, weight):
    if "nc" not in _cached:
        _cached["nc"] = _build()
    nc = _cached["nc"]
    in_maps = _shard_inputs(x, adj, t, weight)
    res = run_bass_kernel_spmd(nc, in_maps, list(range(C)))
    return _assemble(res)
